# revision 2
# baseline (speedup 1.0000x reference)
"""CLoRALinear Trainium2 kernel.

Computes y = x @ (W + (alpha/r) * A @ B.T).T + bias for
x:[4,2048,4096] f32, W:[4096,4096], bias:[4096], A:[4096,32], B:[4096,32].

Strategy: data-parallel over tokens across 8 NeuronCores (1024 tokens each).
Per core, in bf16 with fp32 PSUM accumulation:
  y_tile[128t, 512o] = sum_k x.T_k[:,m].T @ W.T_k[:,n]   (32 k-tiles)
                     + u_aug[:,m].T @ A_aug[:,n]          (LoRA + bias, K=33)
where u_aug rows 0:32 = (x @ B).T and row 32 = 1.0; A_aug rows 0:32 = A.T and
row 32 = bias.  x.T / W.T tiles are produced on-chip via PE transposes
(fp32 has no DMA-transpose path); fp32->bf16 casts ride the SWDGE DMAs.
alpha/r == 1.0, so no scale factor is applied.
"""

import sys

sys.path.insert(0, "/opt/trn_rl_repo")

import numpy as np

import concourse.bass as bass
import concourse.tile as tile
from concourse import bacc, mybir
from concourse.bass_utils import run_bass_kernel_spmd
from concourse.masks import make_identity

F32 = mybir.dt.float32
BF16 = mybir.dt.bfloat16

N_CORES = 8
TOK = 1024          # tokens per core
DIN = 4096
DOUT = 4096
R = 32
KT = DIN // 128     # 32 k-tiles
MT = TOK // 128     # 8 m-tiles
NSL = 512           # out-features per n-slice
NT = DOUT // NSL    # 8 n-slices

_cached = None


def _build():
    nc = bacc.Bacc("TRN2", target_bir_lowering=False, debug=False)

    x_d = nc.dram_tensor("x", [TOK, DIN], F32, kind="ExternalInput").ap()
    w_d = nc.dram_tensor("weight", [DOUT, DIN], F32, kind="ExternalInput").ap()
    bias_d = nc.dram_tensor("bias", [DOUT], F32, kind="ExternalInput").ap()
    a_d = nc.dram_tensor("A", [DOUT, R], F32, kind="ExternalInput").ap()
    b_d = nc.dram_tensor("B", [DIN, R], F32, kind="ExternalInput").ap()
    y_d = nc.dram_tensor("out", [TOK, DOUT], F32, kind="ExternalOutput").ap()

    with tile.TileContext(nc) as tc:
        with (
            tc.tile_pool(name="const", bufs=1) as const_pool,
            tc.tile_pool(name="chunks", bufs=3) as chunk_pool,
            tc.tile_pool(name="wT", bufs=2) as wT_pool,
            tc.tile_pool(name="yout", bufs=3) as y_pool,
            tc.tile_pool(name="tpsum", bufs=3, space="PSUM") as tpsum_pool,
            tc.tile_pool(name="ypsum", bufs=2, space="PSUM") as ypsum_pool,
            tc.tile_pool(name="upsum", bufs=2, space="PSUM") as upsum_pool,
        ):
            ident = const_pool.tile([128, 128], BF16)
            make_identity(nc, ident[:])

            # B (bf16, natural layout): [128, kt, 32]
            b_all = const_pool.tile([128, KT, R], BF16)
            for k in range(KT):
                nc.gpsimd.dma_start(b_all[:, k, :], b_d[k * 128:(k + 1) * 128, :])

            # A_aug: rows 0:32 = A.T, row 32 = bias
            a_aug = const_pool.tile([R + 1, DOUT], BF16)
            nc.gpsimd.dma_start(a_aug[R:R + 1, :], bias_d[None, :])
            for o in range(DOUT // 128):
                a_chunk = chunk_pool.tile([128, R], BF16, tag="a_chunk")
                nc.gpsimd.dma_start(a_chunk[:], a_d[o * 128:(o + 1) * 128, :])
                pt = tpsum_pool.tile([R, 128], BF16, tag="t")
                nc.tensor.transpose(pt[:], a_chunk[:], ident[:])
                nc.vector.tensor_copy(a_aug[0:R, o * 128:(o + 1) * 128], pt[:])

            # x.T resident: [128, kt, tok], plus u_aug = [(x@B).T ; ones]
            x_t = const_pool.tile([128, KT, TOK], BF16)
            u_aug = const_pool.tile([R + 1, TOK], BF16)
            nc.gpsimd.memset(u_aug[R:R + 1, :], 1.0)
            for m in range(MT):
                x_chunk = chunk_pool.tile([128, DIN], BF16, tag="chunk")
                nc.gpsimd.dma_start(x_chunk[:], x_d[m * 128:(m + 1) * 128, :])
                for k in range(KT):
                    pt = tpsum_pool.tile([128, 128], BF16, tag="t")
                    nc.tensor.transpose(
                        pt[:], x_chunk[:, k * 128:(k + 1) * 128], ident[:]
                    )
                    nc.vector.tensor_copy(
                        x_t[:, k, m * 128:(m + 1) * 128], pt[:]
                    )
                up = upsum_pool.tile([R, 128], F32, tag="u")
                for k in range(KT):
                    nc.tensor.matmul(
                        up[:],
                        b_all[:, k, :],
                        x_t[:, k, m * 128:(m + 1) * 128],
                        start=(k == 0),
                        stop=(k == KT - 1),
                    )
                nc.vector.tensor_copy(u_aug[0:R, m * 128:(m + 1) * 128], up[:])

            # Main loop over output-feature slices
            for n in range(NT):
                w_t = wT_pool.tile([128, KT, NSL], BF16)
                for c in range(NSL // 128):
                    w_chunk = chunk_pool.tile([128, DIN], BF16, tag="chunk")
                    nc.gpsimd.dma_start(
                        w_chunk[:],
                        w_d[n * NSL + c * 128:n * NSL + (c + 1) * 128, :],
                    )
                    for k in range(KT):
                        pt = tpsum_pool.tile([128, 128], BF16, tag="t")
                        nc.tensor.transpose(
                            pt[:], w_chunk[:, k * 128:(k + 1) * 128], ident[:]
                        )
                        nc.vector.tensor_copy(
                            w_t[:, k, c * 128:(c + 1) * 128], pt[:]
                        )

                for m in range(MT):
                    yp = ypsum_pool.tile([128, NSL], F32, tag="y")
                    for k in range(KT):
                        nc.tensor.matmul(
                            yp[:],
                            x_t[:, k, m * 128:(m + 1) * 128],
                            w_t[:, k, :],
                            start=(k == 0),
                            stop=False,
                        )
                    nc.tensor.matmul(
                        yp[:],
                        u_aug[:, m * 128:(m + 1) * 128],
                        a_aug[:, n * NSL:(n + 1) * NSL],
                        start=False,
                        stop=True,
                        skip_group_check=True,
                    )
                    y_sb = y_pool.tile([128, NSL], F32, tag="ysb")
                    nc.scalar.copy(y_sb[:], yp[:])
                    nc.sync.dma_start(
                        y_d[m * 128:(m + 1) * 128, n * NSL:(n + 1) * NSL],
                        y_sb[:],
                    )

    nc.compile()
    return nc


def _get_nc():
    global _cached
    if _cached is None:
        _cached = _build()
    return _cached


def kernel(x, weight, bias, A, B, _trace=False):
    x = np.ascontiguousarray(np.asarray(x, dtype=np.float32)).reshape(-1, DIN)
    weight = np.ascontiguousarray(np.asarray(weight, dtype=np.float32))
    bias = np.ascontiguousarray(np.asarray(bias, dtype=np.float32))
    A = np.ascontiguousarray(np.asarray(A, dtype=np.float32))
    B = np.ascontiguousarray(np.asarray(B, dtype=np.float32))

    nc = _get_nc()
    in_maps = [
        {
            "x": np.ascontiguousarray(x[c * TOK:(c + 1) * TOK]),
            "weight": weight,
            "bias": bias,
            "A": A,
            "B": B,
        }
        for c in range(N_CORES)
    ]
    res = run_bass_kernel_spmd(
        nc, in_maps, core_ids=list(range(N_CORES)), trace=_trace
    )
    kernel.last_result = res
    y = np.concatenate([res.results[c]["out"] for c in range(N_CORES)], axis=0)
    return y.reshape(4, 2048, DOUT)


kernel.last_result = None
